# revision 20
# baseline (speedup 1.0000x reference)
"""Trainium2 Bass kernel for a padded/ragged multi-head attention block.

Reference computation (per batch b, full fp32):
    qkv = x[b] @ Wqkv.T ; q,k,v = split(qkv)
    scores = q @ k.T / sqrt(D), key-masked to seq_lengths[b]
    out[b] = softmax(scores) @ v @ Wout.T

Sharding: 8 cores = 4 batches x 2 head-groups of 8 heads. Each core
computes its batch's qkv projection for its 8 heads, full attention for
those heads over all 2048 queries, and a partial out-projection
(contracting only its 512 head-dims). The host sums the two partial
outputs per batch (the tensor-parallel reduce of the unshard step).

Perf design: the kernel is ACT(exp)-bound in attention, so the stream
is arranged to keep the PE continuously busy (the HAM clock gate halves
the PE clock if it idles through its activity window):
  - all matmuls run in fp16 (1 cycle/row streaming, fast weight load);
  - scores for the two heads of a pair run CONCURRENTLY in the top and
    bottom halves of the PE array (row tiling via base partition);
  - the qkv projection of later head-pairs and the out-projection are
    sliced into ~1-2us micro-units and woven between the kt iterations
    of earlier attention blocks as PE filler;
  - attention for pair 0 / query-block 0 is interleaved with the
    startup projection waves, which chase the x DMA stream;
  - pair 3's out-projection lags one query block so it never waits on
    the normalize chain (whose reciprocal is slow on the DVE).

Softmax denominator rides as a 65th ones-column through the attn@v
matmul; normalization is reciprocal + gpsimd partition-broadcast + mul.

Ragged handling: V rows (and the ones-column) are zeroed for masked
keys, so masked keys contribute to neither numerator nor denominator.
exp() needs no max-subtraction: scores are O(6) for these input stats.
The number of 128-wide key tiles is baked at build time from
max(seq_lengths); the per-core mask handles the rest.
"""

import math
from contextlib import ExitStack

import numpy as np

import concourse.bass as bass
import concourse.mybir as mybir
import concourse.tile as tile
from concourse import bacc
from concourse.bass_utils import run_bass_kernel_spmd

F32 = mybir.dt.float32
F16 = mybir.dt.float16
EXP = mybir.ActivationFunctionType.Exp

B, S, E, H, D = 4, 2048, 1024, 16, 64
NCORES = 8
HL = H // 2            # heads per core (8)
EL = HL * D            # embed dims per core (512)
ST = S // 128          # max key tiles (16)
QB = S // 512          # 4 query blocks
EC = E // 128          # 8 contraction chunks

_NC_CACHE: dict[int, object] = {}


def build_nc(nk: int):
    """Build the SPMD program with nk key-tiles (nk*128 keys attended)."""
    nc = bacc.Bacc("TRN2", target_bir_lowering=False, debug=False)

    xT = nc.dram_tensor("xT", [E, S], F16, kind="ExternalInput")
    # weights arrive pre-tiled by the host so every DMA is a contiguous
    # per-partition run (the naive (c p) n -> p c n DMA is descriptor-bound)
    wqk = nc.dram_tensor("wqk", [8, 128, EC * 128], F16, kind="ExternalInput")
    wv2 = nc.dram_tensor("wv2", [2, 128, EC * 256], F16, kind="ExternalInput")
    woutT = nc.dram_tensor("woutT", [128, 4 * E], F16, kind="ExternalInput")
    kmask = nc.dram_tensor("kmask", [128, nk], F32, kind="ExternalInput")
    outp = nc.dram_tensor("outp", [S, E], F32, kind="ExternalOutput")

    with tile.TileContext(nc) as tc, ExitStack() as ctx:
        xpool = ctx.enter_context(tc.tile_pool(name="xp", bufs=1))
        qpool = ctx.enter_context(tc.tile_pool(name="qp", bufs=1))
        kpool = ctx.enter_context(tc.tile_pool(name="kp", bufs=1))
        vpool = ctx.enter_context(tc.tile_pool(name="vp", bufs=1))
        apool = ctx.enter_context(tc.tile_pool(name="ap", bufs=1))
        wpool = ctx.enter_context(tc.tile_pool(name="wp", bufs=3))
        wopool = ctx.enter_context(tc.tile_pool(name="wo", bufs=1))
        ptpool = ctx.enter_context(tc.tile_pool(name="pt", bufs=4))
        czpool = ctx.enter_context(tc.tile_pool(name="cz", bufs=2))
        rdpool = ctx.enter_context(tc.tile_pool(name="rd", bufs=2))
        bcpool = ctx.enter_context(tc.tile_pool(name="bc", bufs=2))
        stpool = ctx.enter_context(tc.tile_pool(name="st", bufs=3))
        kmpool = ctx.enter_context(tc.tile_pool(name="km", bufs=1))

        pspool = ctx.enter_context(tc.tile_pool(name="ps", bufs=2, space="PSUM"))
        scpool = ctx.enter_context(tc.tile_pool(name="sc", bufs=2, space="PSUM"))
        atpool = ctx.enter_context(tc.tile_pool(name="at", bufs=1, space="PSUM"))

        # ---- persistent SBUF tensors ----
        xsb = xpool.tile([128, EC, S], F16)         # x^T
        qsb = qpool.tile([128, 4, S], F16)          # q^T  [pair-dims, pair, seq]
        ksb = kpool.tile([128, 4, S], F16)          # k^T
        vsb = vpool.tile([128, nk, HL, 65], F16)    # v (+ones col), mask folded
        aosb = apool.tile([128, 4, S], F16)         # normalized attention output
        kmsb = kmpool.tile([128, nk], F32)
        wosb = wopool.tile([128, 4, E], F16)

        # ---- x first on the DMA queues, then kmask ----
        for c in range(EC):
            nc.sync.dma_start(xsb[:, c, :], xT.ap()[c * 128 : (c + 1) * 128, :])
        nc.sync.dma_start(kmsb[:], kmask.ap())

        # ---- micro-unit emitters (each ~1-2us of PE work) ----
        def w_dma(seg, p):
            """DMA one pre-tiled 128-col slice of Wqkv^T (q/k, head-pair p)."""
            wt = wpool.tile([128, EC, 128], F16, tag="w")
            nc.sync.dma_start(
                wt[:],
                wqk.ap()[4 * seg + p].rearrange("p (c n) -> p c n", n=128),
            )
            return wt

        def wv_dma(half):
            wv = wpool.tile([128, EC, 256], F16, tag="w")
            nc.sync.dma_start(
                wv[:], wv2.ap()[half].rearrange("p (c n) -> p c n", n=256)
            )
            return wv

        def qk_proj_half(box, wt, p, dest, sb, half):
            """Half of one seq-block of the q/k projection (4 MMs [+copy])."""
            if half == 0:
                box["ps"] = pspool.tile([128, 512], F32, tag="ps", name="ps")
            ps = box["ps"]
            for ec in range(half * 4, half * 4 + 4):
                nc.tensor.matmul(
                    ps[:],
                    lhsT=wt[:, ec, 0:128],
                    rhs=xsb[:, ec, sb * 512 : (sb + 1) * 512],
                    start=(ec == 0),
                    stop=(ec == EC - 1),
                )
            if half == 1:
                nc.vector.tensor_copy(dest[:, p, sb * 512 : (sb + 1) * 512], ps[:])

        def qk_proj_sb(wt, p, dest, sb):
            box = {}
            qk_proj_half(box, wt, p, dest, sb, 0)
            qk_proj_half(box, wt, p, dest, sb, 1)

        def v_proj_part(box, wv, half, st, part):
            if part == 0:
                box["ps"] = pspool.tile([128, 512], F32, tag="ps", name="ps")
            ps = box["ps"]
            for ec in range(part * 4, part * 4 + 4):
                nc.tensor.matmul(
                    ps[:, 0:256],
                    lhsT=xsb[:, ec, st * 128 : (st + 1) * 128],
                    rhs=wv[:, ec, :],
                    start=(ec == 0),
                    stop=(ec == EC - 1),
                )
            if part == 1:
                nc.vector.tensor_scalar_mul(
                    vsb[:, st, half * 4 : (half + 1) * 4, 0:64],
                    ps[:, 0:256].rearrange("p (h d) -> p h d", d=64),
                    kmsb[:, st : st + 1],
                )

        def v_proj_st(wv, half, st):
            box = {}
            v_proj_part(box, wv, half, st, 0)
            v_proj_part(box, wv, half, st, 1)

        def v_ones(half):
            for hl in range(half * 4, (half + 1) * 4):
                nc.vector.tensor_copy(vsb[:, 0:nk, hl, 64], kmsb[:, 0:nk])

        def wout_dma():
            nc.sync.dma_start(wosb[:], woutT.ap().rearrange("p (c n) -> p c n", n=E))

        def outproj_half(box, fb, qt, half):
            if half == 0:
                box["ps"] = pspool.tile([128, 512], F32, tag="ps", name="ps")
            ps = box["ps"]
            for c in range(half * 2, half * 2 + 2):
                nc.tensor.matmul(
                    ps[:],
                    lhsT=aosb[:, c, qt * 128 : (qt + 1) * 128],
                    rhs=wosb[:, c, fb * 512 : (fb + 1) * 512],
                    start=(c == 0),
                    stop=(c == 3),
                )
            if half == 1:
                stg = stpool.tile([128, 512], F32, tag="st")
                nc.vector.tensor_copy(stg[:], ps[:])
                nc.sync.dma_start(
                    outp.ap()[qt * 128 : (qt + 1) * 128, fb * 512 : (fb + 1) * 512],
                    stg[:],
                )

        def outproj_unit(fb, qt):
            box = {}
            outproj_half(box, fb, qt, 0)
            outproj_half(box, fb, qt, 1)

        def qk_proj_units(seg, p, dest):
            wt_box = {}

            def dma_unit():
                wt_box["wt"] = w_dma(seg, p)

            units = [dma_unit]
            for sb in range(4):
                box = {}
                units.append(lambda sb=sb, box=box: qk_proj_half(
                    box, wt_box["wt"], p, dest, sb, 0))
                units.append(lambda sb=sb, box=box: qk_proj_half(
                    box, wt_box["wt"], p, dest, sb, 1))
            return units

        def v_proj_units(half, st_list, with_dma, with_ones):
            def dma_unit():
                v_proj_units_live[half] = wv_dma(half)

            units = [dma_unit] if with_dma else []
            for st in st_list:
                box = {}
                for part in range(2):
                    units.append(lambda st=st, box=box, part=part: v_proj_part(
                        box, v_proj_units_live[half], half, st, part))
            if with_ones:
                units.append(lambda: v_ones(half))
            return units

        v_proj_units_live = {}

        # ---- one attention block (pair p, query block qb) ----
        def attn_block(p, qb, filler, mode="spread", wave=None):
            """Emit attention block (p, qb), weaving filler micro-units
            between kt iterations. mode 'spread': evenly from kt 1 (for
            fillers whose producers are long done); 'tail': packed into the
            last iterations (for fillers depending on the previous block's
            normalize). wave: called before each kt iteration (startup)."""
            q0 = qb * 512
            at2 = atpool.tile([65, 2, 512], F32)
            fill_at = {}
            n = len(filler)
            for j, u in enumerate(filler):
                if mode == "tail":
                    idx = nk - n + j
                else:
                    idx = 1 + (j * max(0, nk - 2)) // max(1, n)
                fill_at.setdefault(max(0, min(nk - 1, idx)), []).append(u)
            for kt in range(nk):
                if wave is not None:
                    wave(kt)
                sc = scpool.tile([128, 2, 512], F32, tag="sc")
                for h2 in range(2):
                    hp = h2 * 64
                    nc.tensor.matmul(
                        sc[:, h2, :],
                        lhsT=ksb[hp : hp + 64, p, kt * 128 : (kt + 1) * 128],
                        rhs=qsb[hp : hp + 64, p, q0 : q0 + 512],
                        start=True,
                        stop=True,
                    )
                pt = ptpool.tile([128, 2, 512], F16, tag="pt")
                nc.scalar.activation(pt[:], sc[:], EXP, scale=1.0 / math.sqrt(D))
                for h2 in range(2):
                    nc.tensor.matmul(
                        at2[0:65, h2, :],
                        lhsT=vsb[:, kt, p * 2 + h2, :],
                        rhs=pt[:, h2, :],
                        start=(kt == 0),
                        stop=(kt == nk - 1),
                    )
                for u in fill_at.get(kt, ()):
                    u()
            return at2

        def normalize(p, qb, at2):
            q0 = qb * 512
            cz = czpool.tile([65, 2, 512], F32, tag="cz")
            nc.vector.tensor_copy(cz[:], at2[0:65, :, :])
            for h2 in range(2):
                rdn = rdpool.tile([1, 512], F32, tag="rd")
                nc.vector.reciprocal(rdn[:], cz[64:65, h2, :])
                bc = bcpool.tile([64, 512], F32, tag="bc")
                nc.gpsimd.partition_broadcast(bc[:], rdn[:])
                nc.vector.tensor_mul(
                    aosb[h2 * 64 : h2 * 64 + 64, p, q0 : q0 + 512],
                    cz[0:64, h2, :],
                    bc[:],
                )

        # ---- emission schedule ----
        # ones columns first: they only need kmsb, and the av matmuls read
        # vsb col 64 from the very first kt iteration.
        v_ones(0)
        v_ones(1)
        # startup: pair-0 projection in per-seq-block waves chasing the x DMA,
        # with attention (pair 0, qb 0) interleaved right behind them.
        wq0 = w_dma(0, 0)
        wk0 = w_dma(1, 0)
        wv0 = wv_dma(0)
        v_proj_units_live[0] = wv0

        wave_done = set()

        def wave(kt):
            """Before attending key-tile kt, ensure projection wave for the
            seq block containing kt (and all earlier blocks) is emitted."""
            for sb in range(4):
                if sb in wave_done or (sb > 0 and sb * 4 > kt):
                    continue
                wave_done.add(sb)
                qk_proj_sb(wq0, 0, qsb, sb)
                qk_proj_sb(wk0, 0, ksb, sb)
                for st in range(sb * 4, min(nk, sb * 4 + 4)):
                    v_proj_st(wv0, 0, st)

        at2 = attn_block(0, 0, [], wave=wave)
        wave(ST)  # flush any waves not triggered when nk is small
        normalize(0, 0, at2)

        # remaining blocks with micro-unit fillers woven in
        plan = {
            (0, 1): qk_proj_units(0, 1, qsb),
            (0, 2): qk_proj_units(1, 1, ksb),
            (0, 3): v_proj_units(1, range(0, 7), with_dma=True, with_ones=False),
            (1, 0): v_proj_units(1, range(7, nk), with_dma=False, with_ones=False),
            (1, 1): qk_proj_units(0, 2, qsb),
            (1, 2): qk_proj_units(1, 2, ksb),
            (1, 3): qk_proj_units(0, 3, qsb),
            (2, 0): qk_proj_units(1, 3, ksb),
            (2, 1): [wout_dma],
            (2, 2): [],
            (2, 3): [],
            (3, 0): [],
            (3, 1): [lambda fb=fb, qt=qt, h=h, box=box: outproj_half(box, fb, qt, h)
                     for fb in range(2) for qt in range(0, 4)
                     for box in ({},) for h in range(2)],
            (3, 2): [lambda fb=fb, qt=qt, h=h, box=box: outproj_half(box, fb, qt, h)
                     for fb in range(2) for qt in range(4, 8)
                     for box in ({},) for h in range(2)],
            (3, 3): [lambda fb=fb, qt=qt, h=h, box=box: outproj_half(box, fb, qt, h)
                     for fb in range(2) for qt in range(8, 12)
                     for box in ({},) for h in range(2)],
        }
        for p in range(4):
            for qb in range(QB):
                if (p, qb) == (0, 0):
                    continue
                mode = "tail" if p == 3 else "spread"
                at2 = attn_block(p, qb, plan[(p, qb)], mode=mode)
                normalize(p, qb, at2)
        for fb in range(2):
            for qt in range(12, 16):
                outproj_unit(fb, qt)

    nc.compile()
    return nc


def make_in_maps(x_padded, seq_lengths, Wqkv, Wout, nk):
    x = np.asarray(x_padded, dtype=np.float32)
    wqkv = np.asarray(Wqkv, dtype=np.float32)
    wout = np.asarray(Wout, dtype=np.float32)
    lens = np.asarray(seq_lengths).astype(np.int64)
    in_maps = []
    for c in range(NCORES):
        b, hg = c // 2, c % 2
        rows = np.concatenate(
            [np.arange(g * E + hg * EL, g * E + (hg + 1) * EL) for g in range(3)]
        )
        km = (np.arange(nk * 128) < int(lens[b])).astype(np.float32)
        km = km.reshape(nk, 128).T
        T = wqkv[rows].T.astype(np.float16)            # [E, 1536]
        # pre-tile: wqk[4*seg+p][part, c*128+n] = T[c*128+part, seg*512+p*128+n]
        qk = np.stack([
            T[:, seg * EL + p * 128 : seg * EL + (p + 1) * 128]
            .reshape(EC, 128, 128).transpose(1, 0, 2).reshape(128, EC * 128)
            for seg in range(2) for p in range(4)
        ])
        vv = np.stack([
            T[:, 2 * EL + h * 256 : 2 * EL + (h + 1) * 256]
            .reshape(EC, 128, 256).transpose(1, 0, 2).reshape(128, EC * 256)
            for h in range(2)
        ])
        W = wout[:, hg * EL : (hg + 1) * EL].T.astype(np.float16)  # [512, E]
        wo = W.reshape(4, 128, E).transpose(1, 0, 2).reshape(128, 4 * E)
        in_maps.append(
            {
                "xT": np.ascontiguousarray(x[b].T.astype(np.float16)),
                "wqk": np.ascontiguousarray(qk),
                "wv2": np.ascontiguousarray(vv),
                "woutT": np.ascontiguousarray(wo),
                "kmask": np.ascontiguousarray(km),
            }
        )
    return in_maps


def kernel(x_padded, seq_lengths, Wqkv, Wout, _profile=None):
    lens = np.asarray(seq_lengths).astype(np.int64)
    nk = int(math.ceil(int(lens.max()) / 128))
    nk = max(1, min(ST, nk))
    if nk not in _NC_CACHE:
        _NC_CACHE[nk] = build_nc(nk)
    nc = _NC_CACHE[nk]

    in_maps = make_in_maps(x_padded, seq_lengths, Wqkv, Wout, nk)
    kwargs = dict(_profile) if _profile else {}
    res = run_bass_kernel_spmd(nc, in_maps, core_ids=list(range(NCORES)), **kwargs)
    if _profile is not None and isinstance(_profile, dict):
        _profile["result"] = res

    out = np.empty((B, S, E), dtype=np.float32)
    for b in range(B):
        out[b] = res.results[2 * b]["outp"] + res.results[2 * b + 1]["outp"]
    return out


# revision 21
# speedup vs baseline: 1.2658x; 1.2658x over previous
"""Trainium2 Bass kernel for a padded/ragged multi-head attention block.

Reference computation (per batch b, full fp32):
    qkv = x[b] @ Wqkv.T ; q,k,v = split(qkv)
    scores = q @ k.T / sqrt(D), key-masked to seq_lengths[b]
    out[b] = softmax(scores) @ v @ Wout.T

Sharding: 8 cores = 4 batches x 2 head-groups of 8 heads. Each core
computes its batch's qkv projection for its 8 heads, full attention for
those heads over all 2048 queries, and a partial out-projection
(contracting only its 512 head-dims). The host sums the two partial
outputs per batch (the tensor-parallel reduce of the unshard step).

Perf design: the kernel is ACT(exp)-bound in attention, so the stream
is arranged to keep the PE continuously busy (the HAM clock gate halves
the PE clock if it idles through its activity window):
  - all matmuls run in fp16 (1 cycle/row streaming, fast weight load);
  - scores for the two heads of a pair run CONCURRENTLY in the top and
    bottom halves of the PE array (row tiling via base partition);
  - the qkv projection of later head-pairs and the out-projection are
    sliced into ~1-2us micro-units and woven between the kt iterations
    of earlier attention blocks as PE filler;
  - attention for pair 0 / query-block 0 is interleaved with the
    startup projection waves, which chase the x DMA stream;
  - pair 3's out-projection lags one query block so it never waits on
    the normalize chain (whose reciprocal is slow on the DVE).

Softmax denominator rides as a 65th ones-column through the attn@v
matmul; normalization is reciprocal + gpsimd partition-broadcast + mul.

Ragged handling: V rows (and the ones-column) are zeroed for masked
keys, so masked keys contribute to neither numerator nor denominator.
exp() needs no max-subtraction: scores are O(6) for these input stats.
The number of 128-wide key tiles is baked at build time from
max(seq_lengths); the per-core mask handles the rest.
"""

import math
from contextlib import ExitStack

import numpy as np

import concourse.bass as bass
import concourse.mybir as mybir
import concourse.tile as tile
from concourse import bacc
from concourse.bass_utils import run_bass_kernel_spmd

F32 = mybir.dt.float32
F16 = mybir.dt.float16
EXP = mybir.ActivationFunctionType.Exp

B, S, E, H, D = 4, 2048, 1024, 16, 64
NCORES = 8
HL = H // 2            # heads per core (8)
EL = HL * D            # embed dims per core (512)
ST = S // 128          # max key tiles (16)
QB = S // 512          # 4 query blocks
EC = E // 128          # 8 contraction chunks

_NC_CACHE: dict[int, object] = {}


def build_nc(nk: int):
    """Build the SPMD program with nk key-tiles (nk*128 keys attended)."""
    nc = bacc.Bacc("TRN2", target_bir_lowering=False, debug=False)

    xT = nc.dram_tensor("xT", [E, S], F16, kind="ExternalInput")
    # weights arrive pre-tiled by the host so every DMA is a contiguous
    # per-partition run (the naive (c p) n -> p c n DMA is descriptor-bound)
    wqk = nc.dram_tensor("wqk", [8, 128, EC * 128], F16, kind="ExternalInput")
    wv2 = nc.dram_tensor("wv2", [2, 128, EC * 256], F16, kind="ExternalInput")
    woutT = nc.dram_tensor("woutT", [128, 4 * E], F16, kind="ExternalInput")
    kmask = nc.dram_tensor("kmask", [128, nk], F32, kind="ExternalInput")
    outp = nc.dram_tensor("outp", [S, E], F32, kind="ExternalOutput")

    with tile.TileContext(nc) as tc, ExitStack() as ctx:
        xpool = ctx.enter_context(tc.tile_pool(name="xp", bufs=1))
        qpool = ctx.enter_context(tc.tile_pool(name="qp", bufs=1))
        kpool = ctx.enter_context(tc.tile_pool(name="kp", bufs=1))
        vpool = ctx.enter_context(tc.tile_pool(name="vp", bufs=1))
        apool = ctx.enter_context(tc.tile_pool(name="ap", bufs=1))
        wpool = ctx.enter_context(tc.tile_pool(name="wp", bufs=3))
        wopool = ctx.enter_context(tc.tile_pool(name="wo", bufs=1))
        ptpool = ctx.enter_context(tc.tile_pool(name="pt", bufs=4))
        czpool = ctx.enter_context(tc.tile_pool(name="cz", bufs=2))
        rdpool = ctx.enter_context(tc.tile_pool(name="rd", bufs=2))
        bcpool = ctx.enter_context(tc.tile_pool(name="bc", bufs=2))
        stpool = ctx.enter_context(tc.tile_pool(name="st", bufs=3))
        kmpool = ctx.enter_context(tc.tile_pool(name="km", bufs=1))

        pspool = ctx.enter_context(tc.tile_pool(name="ps", bufs=2, space="PSUM"))
        scpool = ctx.enter_context(tc.tile_pool(name="sc", bufs=2, space="PSUM"))
        atpool = ctx.enter_context(tc.tile_pool(name="at", bufs=1, space="PSUM"))

        # ---- persistent SBUF tensors ----
        xsb = xpool.tile([128, EC, S], F16)         # x^T
        qsb = qpool.tile([128, 4, S], F16)          # q^T  [pair-dims, pair, seq]
        ksb = kpool.tile([128, 4, S], F16)          # k^T
        vsb = vpool.tile([128, nk, HL, 65], F16)    # v (+ones col), mask folded
        aosb = apool.tile([128, 4, S], F16)         # normalized attention output
        kmsb = kmpool.tile([128, nk], F32)
        wosb = wopool.tile([128, 4, E], F16)

        # ---- x first on the DMA queues, then kmask ----
        for c in range(EC):
            nc.sync.dma_start(xsb[:, c, :], xT.ap()[c * 128 : (c + 1) * 128, :])
        nc.sync.dma_start(kmsb[:], kmask.ap())

        # ---- micro-unit emitters (each ~1-2us of PE work) ----
        def w_dma(seg, p):
            """DMA one pre-tiled 128-col slice of Wqkv^T (q/k, head-pair p)."""
            wt = wpool.tile([128, EC, 128], F16, tag="w")
            nc.sync.dma_start(
                wt[:],
                wqk.ap()[4 * seg + p].rearrange("p (c n) -> p c n", n=128),
            )
            return wt

        def wv_dma(half):
            wv = wpool.tile([128, EC, 256], F16, tag="w")
            nc.sync.dma_start(
                wv[:], wv2.ap()[half].rearrange("p (c n) -> p c n", n=256)
            )
            return wv

        def qk_proj_half(box, wt, p, dest, sb, half):
            """Half of one seq-block of the q/k projection (4 MMs [+copy])."""
            if half == 0:
                box["ps"] = pspool.tile([128, 512], F32, tag="ps", name="ps")
            ps = box["ps"]
            for ec in range(half * 4, half * 4 + 4):
                nc.tensor.matmul(
                    ps[:],
                    lhsT=wt[:, ec, 0:128],
                    rhs=xsb[:, ec, sb * 512 : (sb + 1) * 512],
                    start=(ec == 0),
                    stop=(ec == EC - 1),
                )
            if half == 1:
                nc.vector.tensor_copy(dest[:, p, sb * 512 : (sb + 1) * 512], ps[:])

        def qk_proj_sb(wt, p, dest, sb):
            box = {}
            qk_proj_half(box, wt, p, dest, sb, 0)
            qk_proj_half(box, wt, p, dest, sb, 1)

        def v_proj_part(box, wv, half, st, part):
            if part == 0:
                box["ps"] = pspool.tile([128, 512], F32, tag="ps", name="ps")
            ps = box["ps"]
            for ec in range(part * 4, part * 4 + 4):
                nc.tensor.matmul(
                    ps[:, 0:256],
                    lhsT=xsb[:, ec, st * 128 : (st + 1) * 128],
                    rhs=wv[:, ec, :],
                    start=(ec == 0),
                    stop=(ec == EC - 1),
                )
            if part == 1:
                nc.vector.tensor_scalar_mul(
                    vsb[:, st, half * 4 : (half + 1) * 4, 0:64],
                    ps[:, 0:256].rearrange("p (h d) -> p h d", d=64),
                    kmsb[:, st : st + 1],
                )

        def v_proj_st(wv, half, st):
            box = {}
            v_proj_part(box, wv, half, st, 0)
            v_proj_part(box, wv, half, st, 1)

        def v_ones(half):
            for hl in range(half * 4, (half + 1) * 4):
                nc.vector.tensor_copy(vsb[:, 0:nk, hl, 64], kmsb[:, 0:nk])

        def wout_dma():
            nc.sync.dma_start(wosb[:], woutT.ap().rearrange("p (c n) -> p c n", n=E))

        def outproj_half(box, fb, qt, half):
            if half == 0:
                box["ps"] = pspool.tile([128, 512], F32, tag="ps", name="ps")
            ps = box["ps"]
            for c in range(half * 2, half * 2 + 2):
                nc.tensor.matmul(
                    ps[:],
                    lhsT=aosb[:, c, qt * 128 : (qt + 1) * 128],
                    rhs=wosb[:, c, fb * 512 : (fb + 1) * 512],
                    start=(c == 0),
                    stop=(c == 3),
                )
            if half == 1:
                stg = stpool.tile([128, 512], F32, tag="st")
                nc.vector.tensor_copy(stg[:], ps[:])
                nc.sync.dma_start(
                    outp.ap()[qt * 128 : (qt + 1) * 128, fb * 512 : (fb + 1) * 512],
                    stg[:],
                )

        def outproj_unit(fb, qt):
            box = {}
            outproj_half(box, fb, qt, 0)
            outproj_half(box, fb, qt, 1)

        def qk_proj_units(seg, p, dest):
            wt_box = {}

            def dma_unit():
                wt_box["wt"] = w_dma(seg, p)

            units = [dma_unit]
            for sb in range(4):
                units.append(lambda sb=sb: qk_proj_sb(wt_box["wt"], p, dest, sb))
            return units

        def v_proj_units(half, st_list, with_dma, with_ones):
            def dma_unit():
                v_proj_units_live[half] = wv_dma(half)

            units = [dma_unit] if with_dma else []
            for st in st_list:
                units.append(
                    lambda st=st: v_proj_st(v_proj_units_live[half], half, st)
                )
            if with_ones:
                units.append(lambda: v_ones(half))
            return units

        v_proj_units_live = {}

        # ---- one attention block (pair p, query block qb) ----
        def attn_block(p, qb, filler, mode="spread", wave=None):
            """Emit attention block (p, qb), weaving filler micro-units
            between kt iterations. mode 'spread': evenly from kt 1 (for
            fillers whose producers are long done); 'tail': packed into the
            last iterations (for fillers depending on the previous block's
            normalize). wave: called before each kt iteration (startup)."""
            q0 = qb * 512
            at2 = atpool.tile([65, 2, 512], F32)
            fill_at = {}
            n = len(filler)
            for j, u in enumerate(filler):
                if mode == "tail":
                    idx = nk - n + j
                else:
                    idx = 1 + (j * max(0, nk - 2)) // max(1, n)
                fill_at.setdefault(max(0, min(nk - 1, idx)), []).append(u)
            for kt in range(nk):
                if wave is not None:
                    wave(kt)
                sc = scpool.tile([128, 2, 512], F32, tag="sc")
                for h2 in range(2):
                    hp = h2 * 64
                    nc.tensor.matmul(
                        sc[:, h2, :],
                        lhsT=ksb[hp : hp + 64, p, kt * 128 : (kt + 1) * 128],
                        rhs=qsb[hp : hp + 64, p, q0 : q0 + 512],
                        start=True,
                        stop=True,
                    )
                pt = ptpool.tile([128, 2, 512], F16, tag="pt")
                nc.scalar.activation(pt[:], sc[:], EXP, scale=1.0 / math.sqrt(D))
                for h2 in range(2):
                    nc.tensor.matmul(
                        at2[0:65, h2, :],
                        lhsT=vsb[:, kt, p * 2 + h2, :],
                        rhs=pt[:, h2, :],
                        start=(kt == 0),
                        stop=(kt == nk - 1),
                    )
                for u in fill_at.get(kt, ()):
                    u()
            return at2

        def normalize(p, qb, at2):
            q0 = qb * 512
            cz = czpool.tile([65, 2, 512], F32, tag="cz")
            nc.vector.tensor_copy(cz[:], at2[0:65, :, :])
            for h2 in range(2):
                rdn = rdpool.tile([1, 512], F32, tag="rd")
                nc.vector.reciprocal(rdn[:], cz[64:65, h2, :])
                bc = bcpool.tile([64, 512], F32, tag="bc")
                nc.gpsimd.partition_broadcast(bc[:], rdn[:])
                nc.vector.tensor_mul(
                    aosb[h2 * 64 : h2 * 64 + 64, p, q0 : q0 + 512],
                    cz[0:64, h2, :],
                    bc[:],
                )

        # ---- emission schedule ----
        # ones columns first: they only need kmsb, and the av matmuls read
        # vsb col 64 from the very first kt iteration.
        v_ones(0)
        v_ones(1)
        # startup: pair-0 projection in per-seq-block waves chasing the x DMA,
        # with attention (pair 0, qb 0) interleaved right behind them.
        wq0 = w_dma(0, 0)
        wk0 = w_dma(1, 0)
        wv0 = wv_dma(0)
        v_proj_units_live[0] = wv0

        wave_done = set()

        def wave(kt):
            """Before attending key-tile kt, ensure projection wave for the
            seq block containing kt (and all earlier blocks) is emitted."""
            for sb in range(4):
                if sb in wave_done or (sb > 0 and sb * 4 > kt):
                    continue
                wave_done.add(sb)
                qk_proj_sb(wq0, 0, qsb, sb)
                qk_proj_sb(wk0, 0, ksb, sb)
                for st in range(sb * 4, min(nk, sb * 4 + 4)):
                    v_proj_st(wv0, 0, st)

        at2 = attn_block(0, 0, [], wave=wave)
        wave(ST)  # flush any waves not triggered when nk is small
        normalize(0, 0, at2)

        # remaining blocks with micro-unit fillers woven in
        plan = {
            (0, 1): qk_proj_units(0, 1, qsb),
            (0, 2): qk_proj_units(1, 1, ksb),
            (0, 3): v_proj_units(1, range(0, 7), with_dma=True, with_ones=False),
            (1, 0): v_proj_units(1, range(7, nk), with_dma=False, with_ones=False),
            (1, 1): qk_proj_units(0, 2, qsb),
            (1, 2): qk_proj_units(1, 2, ksb),
            (1, 3): qk_proj_units(0, 3, qsb),
            (2, 0): qk_proj_units(1, 3, ksb),
            (2, 1): [wout_dma],
            (2, 2): [],
            (2, 3): [],
            (3, 0): [],
            (3, 1): [lambda fb=fb, qt=qt: outproj_unit(fb, qt)
                     for fb in range(2) for qt in range(0, 4)],
            (3, 2): [lambda fb=fb, qt=qt: outproj_unit(fb, qt)
                     for fb in range(2) for qt in range(4, 8)],
            (3, 3): [lambda fb=fb, qt=qt: outproj_unit(fb, qt)
                     for fb in range(2) for qt in range(8, 12)],
        }
        for p in range(4):
            for qb in range(QB):
                if (p, qb) == (0, 0):
                    continue
                mode = "tail" if p == 3 else "spread"
                at2 = attn_block(p, qb, plan[(p, qb)], mode=mode)
                normalize(p, qb, at2)
        for fb in range(2):
            for qt in range(12, 16):
                outproj_unit(fb, qt)

    nc.compile()
    return nc


def make_in_maps(x_padded, seq_lengths, Wqkv, Wout, nk):
    x = np.asarray(x_padded, dtype=np.float32)
    wqkv = np.asarray(Wqkv, dtype=np.float32)
    wout = np.asarray(Wout, dtype=np.float32)
    lens = np.asarray(seq_lengths).astype(np.int64)
    in_maps = []
    for c in range(NCORES):
        b, hg = c // 2, c % 2
        rows = np.concatenate(
            [np.arange(g * E + hg * EL, g * E + (hg + 1) * EL) for g in range(3)]
        )
        km = (np.arange(nk * 128) < int(lens[b])).astype(np.float32)
        km = km.reshape(nk, 128).T
        T = wqkv[rows].T.astype(np.float16)            # [E, 1536]
        # pre-tile: wqk[4*seg+p][part, c*128+n] = T[c*128+part, seg*512+p*128+n]
        qk = np.stack([
            T[:, seg * EL + p * 128 : seg * EL + (p + 1) * 128]
            .reshape(EC, 128, 128).transpose(1, 0, 2).reshape(128, EC * 128)
            for seg in range(2) for p in range(4)
        ])
        vv = np.stack([
            T[:, 2 * EL + h * 256 : 2 * EL + (h + 1) * 256]
            .reshape(EC, 128, 256).transpose(1, 0, 2).reshape(128, EC * 256)
            for h in range(2)
        ])
        W = wout[:, hg * EL : (hg + 1) * EL].T.astype(np.float16)  # [512, E]
        wo = W.reshape(4, 128, E).transpose(1, 0, 2).reshape(128, 4 * E)
        in_maps.append(
            {
                "xT": np.ascontiguousarray(x[b].T.astype(np.float16)),
                "wqk": np.ascontiguousarray(qk),
                "wv2": np.ascontiguousarray(vv),
                "woutT": np.ascontiguousarray(wo),
                "kmask": np.ascontiguousarray(km),
            }
        )
    return in_maps


def kernel(x_padded, seq_lengths, Wqkv, Wout, _profile=None):
    lens = np.asarray(seq_lengths).astype(np.int64)
    nk = int(math.ceil(int(lens.max()) / 128))
    nk = max(1, min(ST, nk))
    if nk not in _NC_CACHE:
        _NC_CACHE[nk] = build_nc(nk)
    nc = _NC_CACHE[nk]

    in_maps = make_in_maps(x_padded, seq_lengths, Wqkv, Wout, nk)
    kwargs = dict(_profile) if _profile else {}
    res = run_bass_kernel_spmd(nc, in_maps, core_ids=list(range(NCORES)), **kwargs)
    if _profile is not None and isinstance(_profile, dict):
        _profile["result"] = res

    out = np.empty((B, S, E), dtype=np.float32)
    for b in range(B):
        out[b] = res.results[2 * b]["outp"] + res.results[2 * b + 1]["outp"]
    return out


# revision 22
# speedup vs baseline: 1.4189x; 1.1209x over previous
"""Trainium2 Bass kernel for a padded/ragged multi-head attention block.

Reference computation (per batch b, full fp32):
    qkv = x[b] @ Wqkv.T ; q,k,v = split(qkv)
    scores = q @ k.T / sqrt(D), key-masked to seq_lengths[b]
    out[b] = softmax(scores) @ v @ Wout.T

Sharding: 8 cores = 4 batches x 2 head-groups of 8 heads. Each core
computes its batch's qkv projection for its 8 heads, full attention for
those heads over all 2048 queries, and a partial out-projection
(contracting only its 512 head-dims). The host sums the two partial
outputs per batch (the tensor-parallel reduce of the unshard step).

Perf design: the kernel is ACT(exp)-bound in attention, so the stream
is arranged to keep the PE continuously busy (the HAM clock gate halves
the PE clock if it idles through its activity window):
  - all matmuls run in fp16 (1 cycle/row streaming, fast weight load);
  - scores for the two heads of a pair run CONCURRENTLY in the top and
    bottom halves of the PE array (row tiling via base partition);
  - the qkv projection of later head-pairs and the out-projection are
    sliced into ~1-2us micro-units and woven between the kt iterations
    of earlier attention blocks as PE filler;
  - attention for pair 0 / query-block 0 is interleaved with the
    startup projection waves, which chase the x DMA stream;
  - pair 3's out-projection lags one query block so it never waits on
    the normalize chain (whose reciprocal is slow on the DVE).

Softmax denominator rides as a 65th ones-column through the attn@v
matmul; normalization is reciprocal + gpsimd partition-broadcast + mul.

Ragged handling: V rows (and the ones-column) are zeroed for masked
keys, so masked keys contribute to neither numerator nor denominator.
exp() needs no max-subtraction: scores are O(6) for these input stats.
The number of 128-wide key tiles is baked at build time from
max(seq_lengths); the per-core mask handles the rest.
"""

import math
from contextlib import ExitStack

import numpy as np

import concourse.bass as bass
import concourse.mybir as mybir
import concourse.tile as tile
from concourse import bacc
from concourse.bass_utils import run_bass_kernel_spmd

F32 = mybir.dt.float32
F16 = mybir.dt.float16
EXP = mybir.ActivationFunctionType.Exp

B, S, E, H, D = 4, 2048, 1024, 16, 64
NCORES = 8
HL = H // 2            # heads per core (8)
EL = HL * D            # embed dims per core (512)
ST = S // 128          # max key tiles (16)
QB = S // 512          # 4 query blocks
EC = E // 128          # 8 contraction chunks

_NC_CACHE: dict[int, object] = {}


def build_nc(nk: int):
    """Build the SPMD program with nk key-tiles (nk*128 keys attended)."""
    nc = bacc.Bacc("TRN2", target_bir_lowering=False, debug=False)

    xT = nc.dram_tensor("xT", [E, S], F16, kind="ExternalInput")
    # weights arrive pre-tiled by the host so every DMA is a contiguous
    # per-partition run (the naive (c p) n -> p c n DMA is descriptor-bound)
    wqk = nc.dram_tensor("wqk", [8, 128, EC * 128], F16, kind="ExternalInput")
    wv2 = nc.dram_tensor("wv2", [2, 128, EC * 256], F16, kind="ExternalInput")
    woutT = nc.dram_tensor("woutT", [128, 4 * E], F16, kind="ExternalInput")
    kmask = nc.dram_tensor("kmask", [128, nk], F32, kind="ExternalInput")
    outp = nc.dram_tensor("outp", [S, E], F32, kind="ExternalOutput")

    with tile.TileContext(nc) as tc, ExitStack() as ctx:
        xpool = ctx.enter_context(tc.tile_pool(name="xp", bufs=1))
        qpool = ctx.enter_context(tc.tile_pool(name="qp", bufs=1))
        kpool = ctx.enter_context(tc.tile_pool(name="kp", bufs=1))
        vpool = ctx.enter_context(tc.tile_pool(name="vp", bufs=1))
        apool = ctx.enter_context(tc.tile_pool(name="ap", bufs=1))
        wpool = ctx.enter_context(tc.tile_pool(name="wp", bufs=3))
        wopool = ctx.enter_context(tc.tile_pool(name="wo", bufs=1))
        ptpool = ctx.enter_context(tc.tile_pool(name="pt", bufs=4))
        czpool = ctx.enter_context(tc.tile_pool(name="cz", bufs=2))
        rdpool = ctx.enter_context(tc.tile_pool(name="rd", bufs=2))
        bcpool = ctx.enter_context(tc.tile_pool(name="bc", bufs=2))
        stpool = ctx.enter_context(tc.tile_pool(name="st", bufs=3))
        kmpool = ctx.enter_context(tc.tile_pool(name="km", bufs=1))

        pspool = ctx.enter_context(tc.tile_pool(name="ps", bufs=2, space="PSUM"))
        scpool = ctx.enter_context(tc.tile_pool(name="sc", bufs=2, space="PSUM"))
        atpool = ctx.enter_context(tc.tile_pool(name="at", bufs=1, space="PSUM"))

        # ---- persistent SBUF tensors ----
        xsb = xpool.tile([128, EC, S], F16)         # x^T
        qsb = qpool.tile([128, 4, S], F16)          # q^T  [pair-dims, pair, seq]
        ksb = kpool.tile([128, 4, S], F16)          # k^T
        vsb = vpool.tile([128, nk, HL, 65], F16)    # v (+ones col), mask folded
        aosb = apool.tile([128, 4, S], F16)         # normalized attention output
        kmsb = kmpool.tile([128, nk], F32)
        wosb = wopool.tile([128, 4, E], F16)

        # ---- x first on the DMA queues, then kmask ----
        for c in range(EC):
            nc.sync.dma_start(xsb[:, c, :], xT.ap()[c * 128 : (c + 1) * 128, :])
        nc.sync.dma_start(kmsb[:], kmask.ap())

        # ---- micro-unit emitters (each ~1-2us of PE work) ----
        def w_dma(seg, p):
            """DMA one pre-tiled 128-col slice of Wqkv^T (q/k, head-pair p)."""
            wt = wpool.tile([128, EC, 128], F16, tag="w")
            nc.sync.dma_start(
                wt[:],
                wqk.ap()[4 * seg + p].rearrange("p (c n) -> p c n", n=128),
            )
            return wt

        def wv_dma(half):
            wv = wpool.tile([128, EC, 256], F16, tag="w")
            nc.sync.dma_start(
                wv[:], wv2.ap()[half].rearrange("p (c n) -> p c n", n=256)
            )
            return wv

        def qk_proj_half(box, wt, p, dest, sb, half):
            """Half of one seq-block of the q/k projection (4 MMs [+copy])."""
            if half == 0:
                box["ps"] = pspool.tile([128, 512], F32, tag="ps", name="ps")
            ps = box["ps"]
            for ec in range(half * 4, half * 4 + 4):
                nc.tensor.matmul(
                    ps[:],
                    lhsT=wt[:, ec, 0:128],
                    rhs=xsb[:, ec, sb * 512 : (sb + 1) * 512],
                    start=(ec == 0),
                    stop=(ec == EC - 1),
                )
            if half == 1:
                nc.vector.tensor_copy(dest[:, p, sb * 512 : (sb + 1) * 512], ps[:])

        def qk_proj_sb(wt, p, dest, sb):
            box = {}
            qk_proj_half(box, wt, p, dest, sb, 0)
            qk_proj_half(box, wt, p, dest, sb, 1)

        def v_proj_part(box, wv, half, st, part):
            if part == 0:
                box["ps"] = pspool.tile([128, 512], F32, tag="ps", name="ps")
            ps = box["ps"]
            for ec in range(part * 4, part * 4 + 4):
                nc.tensor.matmul(
                    ps[:, 0:256],
                    lhsT=xsb[:, ec, st * 128 : (st + 1) * 128],
                    rhs=wv[:, ec, :],
                    start=(ec == 0),
                    stop=(ec == EC - 1),
                )
            if part == 1:
                nc.vector.tensor_scalar_mul(
                    vsb[:, st, half * 4 : (half + 1) * 4, 0:64],
                    ps[:, 0:256].rearrange("p (h d) -> p h d", d=64),
                    kmsb[:, st : st + 1],
                )

        def v_proj_st(wv, half, st):
            box = {}
            v_proj_part(box, wv, half, st, 0)
            v_proj_part(box, wv, half, st, 1)

        def v_ones(half):
            for hl in range(half * 4, (half + 1) * 4):
                nc.vector.tensor_copy(vsb[:, 0:nk, hl, 64], kmsb[:, 0:nk])

        def wout_dma():
            nc.sync.dma_start(wosb[:], woutT.ap().rearrange("p (c n) -> p c n", n=E))

        def outproj_half(box, fb, qt, half):
            if half == 0:
                box["ps"] = pspool.tile([128, 512], F32, tag="ps", name="ps")
            ps = box["ps"]
            for c in range(half * 2, half * 2 + 2):
                nc.tensor.matmul(
                    ps[:],
                    lhsT=aosb[:, c, qt * 128 : (qt + 1) * 128],
                    rhs=wosb[:, c, fb * 512 : (fb + 1) * 512],
                    start=(c == 0),
                    stop=(c == 3),
                )
            if half == 1:
                stg = stpool.tile([128, 512], F32, tag="st")
                nc.vector.tensor_copy(stg[:], ps[:])
                nc.sync.dma_start(
                    outp.ap()[qt * 128 : (qt + 1) * 128, fb * 512 : (fb + 1) * 512],
                    stg[:],
                )

        def outproj_unit(fb, qt):
            box = {}
            outproj_half(box, fb, qt, 0)
            outproj_half(box, fb, qt, 1)

        def qk_proj_units(seg, p, dest):
            wt_box = {}

            def dma_unit():
                wt_box["wt"] = w_dma(seg, p)

            units = [dma_unit]
            for sb in range(4):
                units.append(lambda sb=sb: qk_proj_sb(wt_box["wt"], p, dest, sb))
            return units

        def v_proj_units(half, st_list, with_dma, with_ones):
            def dma_unit():
                v_proj_units_live[half] = wv_dma(half)

            units = [dma_unit] if with_dma else []
            for st in st_list:
                units.append(
                    lambda st=st: v_proj_st(v_proj_units_live[half], half, st)
                )
            if with_ones:
                units.append(lambda: v_ones(half))
            return units

        v_proj_units_live = {}

        # ---- one attention block (pair p, query block qb) ----
        def attn_block(p, qb, filler, mode="spread", wave=None):
            """Emit attention block (p, qb), weaving filler micro-units
            between kt iterations. mode 'spread': evenly from kt 1 (for
            fillers whose producers are long done); 'tail': packed into the
            last iterations (for fillers depending on the previous block's
            normalize). wave: called before each kt iteration (startup)."""
            q0 = qb * 512
            at2 = atpool.tile([65, 2, 512], F32)
            fill_at = {}
            n = len(filler)
            for j, u in enumerate(filler):
                if mode == "tail":
                    idx = nk - n + j
                else:
                    idx = 1 + (j * max(0, nk - 2)) // max(1, n)
                fill_at.setdefault(max(0, min(nk - 1, idx)), []).append(u)
            for kt in range(nk):
                if wave is not None:
                    wave(kt)
                sc = scpool.tile([128, 2, 512], F32, tag="sc")
                for h2 in range(2):
                    hp = h2 * 64
                    nc.tensor.matmul(
                        sc[:, h2, :],
                        lhsT=ksb[hp : hp + 64, p, kt * 128 : (kt + 1) * 128],
                        rhs=qsb[hp : hp + 64, p, q0 : q0 + 512],
                        start=True,
                        stop=True,
                    )
                pt = ptpool.tile([128, 2, 512], F16, tag="pt")
                nc.scalar.activation(pt[:], sc[:], EXP, scale=1.0 / math.sqrt(D))
                for h2 in range(2):
                    nc.tensor.matmul(
                        at2[0:65, h2, :],
                        lhsT=vsb[:, kt, p * 2 + h2, :],
                        rhs=pt[:, h2, :],
                        start=(kt == 0),
                        stop=(kt == nk - 1),
                    )
                for u in fill_at.get(kt, ()):
                    u()
            return at2

        def normalize(p, qb, at2):
            q0 = qb * 512
            cz = czpool.tile([65, 2, 512], F32, tag="cz")
            nc.vector.tensor_copy(cz[:], at2[0:65, :, :])
            # spread the 1024 denominators across 128 partitions via a tiny
            # sbuf->sbuf DMA so the reciprocal runs 8 elems/lane instead of
            # 512 on one lane (6.6us of serial DVE -> ~0.3us).
            denT = rdpool.tile([128, 8], F32, tag="dnt", name="denT")
            nc.sync.dma_start(denT[:], cz[64:65, :, :])
            rdnT = rdpool.tile([128, 8], F32, tag="rdt", name="rdnT")
            nc.vector.reciprocal(rdnT[:], denT[:])
            for h2 in range(2):
                rdn = rdpool.tile([1, 512], F32, tag="rd")
                nc.sync.dma_start(rdn[:], rdnT[h2 * 64 : h2 * 64 + 64, :])
                bc = bcpool.tile([64, 512], F32, tag="bc")
                nc.gpsimd.partition_broadcast(bc[:], rdn[:])
                nc.vector.tensor_mul(
                    aosb[h2 * 64 : h2 * 64 + 64, p, q0 : q0 + 512],
                    cz[0:64, h2, :],
                    bc[:],
                )

        # ---- emission schedule ----
        # ones columns first: they only need kmsb, and the av matmuls read
        # vsb col 64 from the very first kt iteration.
        v_ones(0)
        v_ones(1)
        # startup: pair-0 projection in per-seq-block waves chasing the x DMA,
        # with attention (pair 0, qb 0) interleaved right behind them.
        wq0 = w_dma(0, 0)
        wk0 = w_dma(1, 0)
        wv0 = wv_dma(0)
        v_proj_units_live[0] = wv0

        wave_done = set()

        def wave(kt):
            """Before attending key-tile kt, ensure projection wave for the
            seq block containing kt (and all earlier blocks) is emitted."""
            for sb in range(4):
                if sb in wave_done or (sb > 0 and sb * 4 > kt):
                    continue
                wave_done.add(sb)
                qk_proj_sb(wq0, 0, qsb, sb)
                qk_proj_sb(wk0, 0, ksb, sb)
                for st in range(sb * 4, min(nk, sb * 4 + 4)):
                    v_proj_st(wv0, 0, st)

        at2 = attn_block(0, 0, [], wave=wave)
        wave(ST)  # flush any waves not triggered when nk is small
        normalize(0, 0, at2)

        # remaining blocks with micro-unit fillers woven in
        plan = {
            (0, 1): qk_proj_units(0, 1, qsb),
            (0, 2): qk_proj_units(1, 1, ksb),
            (0, 3): v_proj_units(1, range(0, 7), with_dma=True, with_ones=False),
            (1, 0): v_proj_units(1, range(7, nk), with_dma=False, with_ones=False),
            (1, 1): qk_proj_units(0, 2, qsb),
            (1, 2): qk_proj_units(1, 2, ksb),
            (1, 3): qk_proj_units(0, 3, qsb),
            (2, 0): qk_proj_units(1, 3, ksb),
            (2, 1): [wout_dma],
            (2, 2): [],
            (2, 3): [],
            (3, 0): [],
            (3, 1): [lambda fb=fb, qt=qt: outproj_unit(fb, qt)
                     for fb in range(2) for qt in range(0, 4)],
            (3, 2): [lambda fb=fb, qt=qt: outproj_unit(fb, qt)
                     for fb in range(2) for qt in range(4, 8)],
            (3, 3): [lambda fb=fb, qt=qt: outproj_unit(fb, qt)
                     for fb in range(2) for qt in range(8, 12)],
        }
        for p in range(4):
            for qb in range(QB):
                if (p, qb) == (0, 0):
                    continue
                mode = "tail" if p == 3 else "spread"
                at2 = attn_block(p, qb, plan[(p, qb)], mode=mode)
                normalize(p, qb, at2)
        for fb in range(2):
            for qt in range(12, 16):
                outproj_unit(fb, qt)

    nc.compile()
    return nc


def make_in_maps(x_padded, seq_lengths, Wqkv, Wout, nk):
    x = np.asarray(x_padded, dtype=np.float32)
    wqkv = np.asarray(Wqkv, dtype=np.float32)
    wout = np.asarray(Wout, dtype=np.float32)
    lens = np.asarray(seq_lengths).astype(np.int64)
    in_maps = []
    for c in range(NCORES):
        b, hg = c // 2, c % 2
        rows = np.concatenate(
            [np.arange(g * E + hg * EL, g * E + (hg + 1) * EL) for g in range(3)]
        )
        km = (np.arange(nk * 128) < int(lens[b])).astype(np.float32)
        km = km.reshape(nk, 128).T
        T = wqkv[rows].T.astype(np.float16)            # [E, 1536]
        # pre-tile: wqk[4*seg+p][part, c*128+n] = T[c*128+part, seg*512+p*128+n]
        qk = np.stack([
            T[:, seg * EL + p * 128 : seg * EL + (p + 1) * 128]
            .reshape(EC, 128, 128).transpose(1, 0, 2).reshape(128, EC * 128)
            for seg in range(2) for p in range(4)
        ])
        vv = np.stack([
            T[:, 2 * EL + h * 256 : 2 * EL + (h + 1) * 256]
            .reshape(EC, 128, 256).transpose(1, 0, 2).reshape(128, EC * 256)
            for h in range(2)
        ])
        W = wout[:, hg * EL : (hg + 1) * EL].T.astype(np.float16)  # [512, E]
        wo = W.reshape(4, 128, E).transpose(1, 0, 2).reshape(128, 4 * E)
        in_maps.append(
            {
                "xT": np.ascontiguousarray(x[b].T.astype(np.float16)),
                "wqk": np.ascontiguousarray(qk),
                "wv2": np.ascontiguousarray(vv),
                "woutT": np.ascontiguousarray(wo),
                "kmask": np.ascontiguousarray(km),
            }
        )
    return in_maps


def kernel(x_padded, seq_lengths, Wqkv, Wout, _profile=None):
    lens = np.asarray(seq_lengths).astype(np.int64)
    nk = int(math.ceil(int(lens.max()) / 128))
    nk = max(1, min(ST, nk))
    if nk not in _NC_CACHE:
        _NC_CACHE[nk] = build_nc(nk)
    nc = _NC_CACHE[nk]

    in_maps = make_in_maps(x_padded, seq_lengths, Wqkv, Wout, nk)
    kwargs = dict(_profile) if _profile else {}
    res = run_bass_kernel_spmd(nc, in_maps, core_ids=list(range(NCORES)), **kwargs)
    if _profile is not None and isinstance(_profile, dict):
        _profile["result"] = res

    out = np.empty((B, S, E), dtype=np.float32)
    for b in range(B):
        out[b] = res.results[2 * b]["outp"] + res.results[2 * b + 1]["outp"]
    return out
